# revision 33
# baseline (speedup 1.0000x reference)
"""Trainium2 Bass kernel for nn_AttentionLayer (Bahdanau-style attention scorer).

Math (per batch b):
    x   = concat([a, broadcast(s)], -1)            # [Tx, Da+Ds]
    h   = relu(x @ W1 + b1)                        # [Tx, H]
    e   = tanh(h @ W2 + b2)                        # [Tx, 1]
    al  = softmax(e, axis=Tx)
    ctx = al^T @ a                                 # [1, Da]

Since e = tanh(.) is in [-1, 1], softmax needs no max subtraction:
    al = exp(e) / sum(exp(e)) is numerically safe in fp32.

Sharding: data-parallel over B across 8 cores (8 batches each).

The kernel is HBM-bound, so `a` ships in mixed precision to cut bytes:
  - aT  (transposed, for the score matmul):  fp8 e4m3, 4.2 MB/core.
    Scores only feed a softmax through tanh; fp8 here costs ~7e-3 rel
    err end-to-end (validated vs the fp32 reference, tolerance 2e-2).
    Batches 0-1 ship as singles (so scoring starts earlier), 2-7 as
    pairs (8 KB per partition row per DMA).
  - a_nat (natural, for the context matmul): bf16, 8.4 MB/core.  The
    last two batches ship as half-tiles so the final context matmuls
    can chase partial arrivals.
  Total 12.6 MB/core vs 16.8 MB for bf16-both.

DMA order = schedule (single Sync HWDGE stream; the DMA engines drain
descriptors in issue order): one small bf16 weight pack, then ALL aT,
then a_nat.  The scores+softmax pipeline completes while a_nat still
streams and the per-batch context matmuls chase the a_nat arrivals.

Score scatter (mm2) places each batch PAIR's e-rows at PSUM partition
bases 0/32/64/96 of ONE [98, Tx] tile via the matmul tile_position
column coordinate, so softmax needs only two full-width ACT passes
(tanh+exp over partitions [0:34] and [64:98]) and two 16-transpose
bursts — the first half's softmax weights are ready long before its
a_nat tiles arrive.  Relus: batches 0-3 slice 0 on ACT (free until the
first tanh), everything else on DVE tensor_scalar.

Context (per batch, a_nat-DMA-paced): ctx = sum_n p_n^T @ a_n as 4-way
column-tiled quads accumulating at PSUM partitions 0/32/64/96; the four
quarters leave PSUM as two 33-partition-wide DVE copies; host sums
quarters and divides by the denominator.

A small PE warm-up burst during the DMA lead-in starts the HAM clock
ramp early (the PE runs at half clock for the first ~10 us of work
regardless; the full-speed window is spent on the score pipeline).

Host-side preprocessing (transpose/cast/shard + final division) is numpy.
"""

import os
import sys

import numpy as np

for _p in ("/opt/trn_rl_repo", "/root/.axon_site/_ro/trn_rl_repo"):
    if os.path.isdir(_p) and _p not in sys.path:
        sys.path.insert(0, _p)

import ml_dtypes  # noqa: E402

import concourse.bacc as bacc  # noqa: E402
import concourse.bass as bass  # noqa: E402
import concourse.mybir as mybir  # noqa: E402
import concourse.tile as tile  # noqa: E402

BF16 = mybir.dt.bfloat16
F8 = mybir.dt.float8e4
F32 = mybir.dt.float32
NPBF16 = ml_dtypes.bfloat16
NPF8 = ml_dtypes.float8_e4m3
AF = mybir.ActivationFunctionType
ALU = mybir.AluOpType
PSUM = bass.MemorySpace.PSUM

NCORES = 8
B, TX, DA, DS, H = 64, 2048, 256, 256, 50
BPC = B // NCORES  # batches per core
NT = TX // 128  # 128-wide time chunks
NTS = TX // 512  # 512-wide time slices
KD = DA // 128  # contraction chunks over Da (and Ds)
NSING = 2  # leading at8 batches shipped as singles
NHALF = 2  # trailing a_nat batches shipped as half-tiles
PW = 34  # partition width of one softmax half (rows 0,1,32,33 live)

# Weight-pack column layout (single bf16 [128, 512] tensor, 1 KB rows).
_B_W1A = 0
_B_W1S = _B_W1A + KD * 64
_B_ST = _B_W1S + KD * H
_B_B1 = _B_ST + KD * BPC
_B_B2 = _B_B1 + 1
_B_W2 = _B_B2 + 1
_B_ID = _B_W2 + BPC * 2
WCOLS = 512


def jcol(bi):
    """Column of pT holding batch bi's weights within its softmax half."""
    return 32 * ((bi // 2) % 2) + bi % 2


def build_nc():
    """Build the (SPMD-identical) single-core Bass program."""
    nc = bacc.Bacc(
        "TRN2", target_bir_lowering=False, debug=False, num_devices=NCORES
    )

    NPAIR = (BPC - NSING) // 2
    at8a = nc.dram_tensor(
        "at8a", [NSING, 128, KD, TX], F8, kind="ExternalInput"
    )
    at8b = nc.dram_tensor(
        "at8b", [NPAIR, 128, 2, KD, TX], F8, kind="ExternalInput"
    )
    a_nat = nc.dram_tensor("a_nat", [BPC, 128, NT, DA], BF16, kind="ExternalInput")
    wpk = nc.dram_tensor("wpk", [128, WCOLS], BF16, kind="ExternalInput")
    ctx_o = nc.dram_tensor("ctx_o", [4, BPC, DA], F32, kind="ExternalOutput")
    den_o = nc.dram_tensor("den_o", [BPC, 1], F32, kind="ExternalOutput")

    with tile.TileContext(nc) as tc:
        with tc.tile_pool(name="const", bufs=1) as cpool, tc.tile_pool(
            name="at8s", bufs=NSING
        ) as atspool, tc.tile_pool(name="at8p", bufs=NPAIR) as atpool, tc.tile_pool(
            name="anat", bufs=BPC - NHALF
        ) as apool, tc.tile_pool(name="anath", bufs=2 * NHALF) as ahpool, tc.tile_pool(
            name="hsb", bufs=BPC
        ) as hsbp, tc.tile_pool(name="sb2", bufs=1) as sb2:
            ats_tiles = [
                atspool.tile([128, KD, TX], F8, name=f"ats{b}", tag="ats")
                for b in range(NSING)
            ]
            atp_tiles = [
                atpool.tile([128, 2, KD, TX], F8, name=f"atp{p}", tag="atp")
                for p in range(NPAIR)
            ]
            # a_nat: full tiles for b < BPC-NHALF, two half-tiles for the rest
            # (half h covers time chunks [8h, 8h+8)).
            a_tiles = {}
            for b in range(BPC - NHALF):
                a_tiles[b] = apool.tile(
                    [128, NT, DA], BF16, name=f"a_t{b}", tag="a_t"
                )
            for b in range(BPC - NHALF, BPC):
                a_tiles[b] = [
                    ahpool.tile([128, NT // 2, DA], BF16, name=f"a_t{b}h{h}",
                                tag="a_th")
                    for h in range(2)
                ]
            w = cpool.tile([128, WCOLS], BF16)
            nc.sync.dma_start(w[:], wpk[:])
            for b in range(NSING):
                nc.sync.dma_start(ats_tiles[b][:], at8a[b])
            for p in range(NPAIR):
                nc.sync.dma_start(atp_tiles[p][:], at8b[p])
            for b in range(BPC - NHALF):
                nc.sync.dma_start(a_tiles[b][:], a_nat[b])
            anat_h = a_nat.reshape([BPC, 128, 2, NT // 2, DA])
            for b in range(BPC - NHALF, BPC):
                for h in range(2):
                    nc.sync.dma_start(a_tiles[b][h][:], anat_h[b, :, h])

            def at_rhs(bi, k, ts):
                """Moving operand for mm1: one 512-wide time slice."""
                sl = slice(ts * 512, (ts + 1) * 512)
                if bi < NSING:
                    return ats_tiles[bi][:, k, sl]
                p, r = divmod(bi - NSING, 2)
                return atp_tiles[p][:, r, k, sl]

            def a_chunk(bi, n):
                """[128, DA] natural-layout chunk n of batch bi."""
                if bi < BPC - NHALF:
                    return a_tiles[bi][:, n, :]
                return a_tiles[bi][n // (NT // 2)][:, n % (NT // 2), :]

            sterm_sb = sb2.tile([128, BPC], F32)
            ctx_sb = sb2.tile([97, BPC, DA], F32)
            t_sb = sb2.tile([98, TX], F32, tag="tsb")
            p_sb = sb2.tile([98, TX], BF16, tag="psb")
            den_sb = sb2.tile([98, 1], F32, tag="den")
            warm_sb = sb2.tile([128, 512], BF16, tag="warm")
            nc.gpsimd.memset(warm_sb[:], 0.0)
            nc.gpsimd.memset(sterm_sb[:], 0.0)

            with tc.tile_pool(name="hps", bufs=2, space=PSUM) as hps, tc.tile_pool(
                name="eps", bufs=1, space=PSUM
            ) as eps, tc.tile_pool(name="p3", bufs=2, space=PSUM) as p3:
                # PE warm-up: short N=128 matmuls keep the PE continuously
                # busy through the DMA lead-in (weight pack + first aT tile)
                # so the HAM clock reaches full speed before the real score
                # matmuls start; otherwise the PE runs at half clock for the
                # first ~10 us of mm1.
                warm_ps = hps.tile([128, 512], F32, tag="hps", name="warm_ps")

                def emit_warm(n):
                    for _ in range(n):
                        nc.tensor.matmul(
                            warm_ps[0:64, 0:128],
                            warm_sb[:, 0:64],
                            warm_sb[:, 0:128],
                            start=True,
                            stop=True,
                            skip_group_check=True,
                        )

                emit_warm(14)

                # s-term, twice: partitions 0-49 (col group 0) and 64-113
                # (col group 64), so both relu halves get a bias.  Both
                # matmul groups land before either identity reads the tile
                # (a reader in between would WAR-stall the second group).
                sterm_ps = hps.tile([128, 512], F32, tag="hps", name="sterm_ps")
                for cg in (0, 64):
                    for k in range(KD):
                        nc.tensor.matmul(
                            sterm_ps[cg : cg + H, 0:BPC],
                            w[:, _B_W1S + k * H : _B_W1S + (k + 1) * H],
                            w[:, _B_ST + k * BPC : _B_ST + (k + 1) * BPC],
                            start=(k == 0),
                            stop=(k == KD - 1),
                            tile_position=(0, cg),
                            skip_group_check=True,
                        )
                for cg in (0, 64):
                    nc.scalar.activation(
                        sterm_sb[cg : cg + H, :],
                        sterm_ps[cg : cg + H, 0:BPC],
                        AF.Identity,
                        bias=w[cg : cg + H, _B_B1 : _B_B1 + 1],
                    )
                emit_warm(8)

                e_ps = eps.tile([98, TX], F32, tag="eps", name="e_ps")

                def emit_mm1(bi):
                    """Score matmuls + relus for one batch (column-tiled
                    512-wide pairs).  Returns the h tiles."""
                    tiles = []
                    for tp in range(NTS // 2):
                        h_ps = hps.tile([128, 512], F32, tag="hps")
                        for k in range(KD):
                            for half, cg in ((0, 0), (1, 64)):
                                ts = 2 * tp + half
                                nc.tensor.matmul(
                                    h_ps[cg : cg + 64, :],
                                    w[:, _B_W1A + k * 64 : _B_W1A + (k + 1) * 64],
                                    at_rhs(bi, k, ts),
                                    start=(k == 0),
                                    stop=(k == KD - 1),
                                    tile_position=(0, cg),
                                    skip_group_check=True,
                                )
                        h_sb = hsbp.tile([128, 512], BF16, tag="hsb")
                        if bi < 4 and tp == 0:
                            nc.scalar.activation(
                                h_sb[:],
                                h_ps[:],
                                AF.Relu,
                                bias=sterm_sb[:, bi : bi + 1],
                            )
                        else:
                            nc.vector.tensor_scalar(
                                h_sb[:],
                                h_ps[:],
                                sterm_sb[:, bi : bi + 1],
                                0.0,
                                ALU.add,
                                ALU.max,
                            )
                        tiles.append(h_sb)
                    return tiles

                def emit_mm2(bi, h_tiles):
                    """Scatter batch bi's scores into e_ps rows base+j via
                    the tile_position column coordinate."""
                    base = 32 * (bi // 2)
                    j = bi % 2
                    for tp in range(NTS // 2):
                        for half, cg in ((0, 0), (1, 64)):
                            ts = 2 * tp + half
                            nc.tensor.matmul(
                                e_ps[base : base + 2, ts * 512 : (ts + 1) * 512],
                                w[cg : cg + H, _B_W2 + bi * 2 : _B_W2 + (bi + 1) * 2],
                                h_tiles[tp][cg : cg + H, :],
                                start=(j == 0),
                                stop=(j == 1),
                                tile_position=(cg, base),
                                skip_group_check=True,
                            )

                def emit_softmax(hi):
                    """tanh -> exp(+den) over one 34-partition half on ACT."""
                    base = 64 * hi
                    sl = slice(base, base + PW)
                    nc.scalar.activation(
                        t_sb[sl, :],
                        e_ps[sl, :],
                        AF.Tanh,
                        bias=w[sl, _B_B2 : _B_B2 + 1],
                    )
                    nc.scalar.activation(
                        p_sb[sl, :],
                        t_sb[sl, :],
                        AF.Exp,
                        accum_out=den_sb[sl, :],
                    )
                    for g in (0, 1):
                        nc.gpsimd.dma_start(
                            den_o[4 * hi + 2 * g : 4 * hi + 2 * g + 2],
                            den_sb[base + 32 * g : base + 32 * g + 2, :],
                        )

                def emit_ptrans(hi):
                    """One 16-transpose burst: p rows [64hi : 64hi+34] ->
                    time-major pT, then a single DVE copy out."""
                    base = 64 * hi
                    pt_ps = p3.tile(
                        [128, NT * PW], BF16, tag="p3", name=f"pt_ps{hi}"
                    )
                    for n in range(NT):
                        nc.tensor.transpose(
                            pt_ps[:, n * PW : (n + 1) * PW],
                            p_sb[base : base + PW, n * 128 : (n + 1) * 128],
                            w[base : base + PW, _B_ID : _B_ID + PW],
                        )
                    pT_sb = sb2.tile([128, NT * PW], BF16, tag=f"pT{hi}")
                    nc.vector.tensor_copy(pT_sb[:], pt_ps[:])
                    return pT_sb

                def emit_ctx(bi, pT_sb):
                    jj = jcol(bi)
                    c_ps = p3.tile([128, DA], F32, tag="p3", name=f"c_ps{bi}")
                    for np_ in range(NT // 4):
                        for qi, cg in enumerate((0, 32, 64, 96)):
                            n = 4 * np_ + qi
                            nc.tensor.matmul(
                                c_ps[cg : cg + 1, :],
                                pT_sb[:, n * PW + jj : n * PW + jj + 1],
                                a_chunk(bi, n),
                                start=(np_ == 0),
                                stop=(np_ == NT // 4 - 1),
                                tile_position=(0, cg),
                                skip_group_check=True,
                            )
                    # Quarters sit at partitions 0/32/64/96; ship them as two
                    # concurrent 33-partition copies (DVE + ACT, which is free
                    # once the second exp is done).
                    nc.vector.tensor_copy(ctx_sb[0:33, bi, :], c_ps[0:33, :])
                    nc.scalar.copy(ctx_sb[64:97, bi, :], c_ps[64:97, :])

                def emit_out(lo):
                    engines = (nc.sync, nc.gpsimd, nc.scalar, nc.gpsimd)
                    for qi, cg in enumerate((0, 32, 64, 96)):
                        engines[qi].dma_start(
                            ctx_o[qi, lo : lo + 4, :],
                            ctx_sb[cg : cg + 1, lo : lo + 4, :],
                        )

                # ---- emission schedule ----
                # mm2 runs one batch behind mm1; each softmax half fires as
                # soon as its last mm2 is in; ctx(0-2) sit between the two
                # transpose bursts so they can chase the first a_nat arrivals.
                h_all = {}
                for bi in range(BPC):
                    h_all[bi] = emit_mm1(bi)
                    if bi > 0:
                        emit_mm2(bi - 1, h_all[bi - 1])
                    if bi == 4:
                        emit_softmax(0)
                emit_mm2(BPC - 1, h_all[BPC - 1])
                emit_softmax(1)
                pT_A = emit_ptrans(0)
                for bi in range(3):
                    emit_ctx(bi, pT_A)
                pT_B = emit_ptrans(1)
                emit_ctx(3, pT_A)
                emit_out(0)
                for bi in range(4, BPC):
                    emit_ctx(bi, pT_B)
                emit_out(4)

    nc.compile()
    return nc


def make_in_maps(a, s, W1, b1, W2, b2):
    a = np.asarray(a, np.float32)
    s = np.asarray(s, np.float32)
    W1 = np.asarray(W1, np.float32)
    b1 = np.asarray(b1, np.float32)
    W2 = np.asarray(W2, np.float32)
    b2 = np.asarray(b2, np.float32)

    NPAIR = (BPC - NSING) // 2
    a5 = a.reshape(NCORES, BPC, TX, DA)
    s3 = s.reshape(NCORES, BPC, DS)

    wpk_base = np.zeros((128, WCOLS), np.float32)
    w1a_full = np.zeros((128, KD, 64), np.float32)
    w1a_full[:, :, :H] = W1[:DA].reshape(KD, 128, H).transpose(1, 0, 2)
    wpk_base[:, _B_W1A : _B_W1A + KD * 64] = w1a_full.reshape(128, KD * 64)
    wpk_base[:, _B_W1S : _B_W1S + KD * H] = (
        W1[DA:].reshape(KD, 128, H).transpose(1, 0, 2).reshape(128, KD * H)
    )
    wpk_base[0:H, _B_B1] = b1
    wpk_base[64 : 64 + H, _B_B1] = b1
    wpk_base[:, _B_B2] = float(b2.reshape(-1)[0])
    oh = np.einsum(
        "h,bm->hbm", W2[:, 0], np.eye(2)[np.arange(BPC) % 2]
    ).reshape(H, BPC * 2)
    wpk_base[0:H, _B_W2 : _B_W2 + BPC * 2] = oh
    wpk_base[64 : 64 + H, _B_W2 : _B_W2 + BPC * 2] = oh
    wpk_base[0:PW, _B_ID : _B_ID + PW] = np.eye(PW)
    wpk_base[64 : 64 + PW, _B_ID : _B_ID + PW] = np.eye(PW)

    in_maps = []
    for i in range(NCORES):
        ai = a5[i]
        a_nat_h = np.ascontiguousarray(
            ai.reshape(BPC, NT, 128, DA).transpose(0, 2, 1, 3)
        ).astype(NPBF16)
        at_all = (
            ai.transpose(0, 2, 1).reshape(BPC, KD, 128, TX).transpose(0, 2, 1, 3)
        )
        at8a_h = np.ascontiguousarray(at_all[:NSING]).astype(NPF8)
        at8b_h = np.ascontiguousarray(
            at_all[NSING:].reshape(NPAIR, 2, 128, KD, TX).transpose(0, 2, 1, 3, 4)
        ).astype(NPF8)
        wpk_h = wpk_base.copy()
        wpk_h[:, _B_ST : _B_ST + KD * BPC] = (
            s3[i].T.reshape(KD, 128, BPC).transpose(1, 0, 2).reshape(128, KD * BPC)
        )
        in_maps.append(
            {
                "at8a": at8a_h,
                "at8b": at8b_h,
                "a_nat": a_nat_h,
                "wpk": wpk_h.astype(NPBF16),
            }
        )
    return in_maps


def assemble_output(results):
    outs = []
    for i in range(NCORES):
        ctx4 = results[i]["ctx_o"].astype(np.float64)
        ctx = ctx4.sum(axis=0)
        den = results[i]["den_o"].astype(np.float64)
        outs.append(ctx / den)
    return np.concatenate(outs, 0).reshape(B, 1, DA).astype(np.float32)


_NC_CACHE = None


def _get_nc():
    global _NC_CACHE
    if _NC_CACHE is None:
        _NC_CACHE = build_nc()
    return _NC_CACHE


def kernel(a, s, W1, b1, W2, b2, trace=False):
    from concourse.bass_utils import run_bass_kernel_spmd

    nc = _get_nc()
    in_maps = make_in_maps(a, s, W1, b1, W2, b2)
    res = run_bass_kernel_spmd(
        nc, in_maps, core_ids=list(range(NCORES)), trace=trace
    )
    out = assemble_output(res.results)
    if trace:
        kernel.last_exec_time_ns = res.exec_time_ns
        kernel.last_results = res
    return out


# revision 34
# speedup vs baseline: 1.0569x; 1.0569x over previous
"""Trainium2 Bass kernel for nn_AttentionLayer (Bahdanau-style attention scorer).

Math (per batch b):
    x   = concat([a, broadcast(s)], -1)            # [Tx, Da+Ds]
    h   = relu(x @ W1 + b1)                        # [Tx, H]
    e   = tanh(h @ W2 + b2)                        # [Tx, 1]
    al  = softmax(e, axis=Tx)
    ctx = al^T @ a                                 # [1, Da]

Since e = tanh(.) is in [-1, 1], softmax needs no max subtraction:
    al = exp(e) / sum(exp(e)) is numerically safe in fp32.

Sharding: data-parallel over B across 8 cores (8 batches each).

The kernel is HBM-bound, so `a` ships in mixed precision to cut bytes:
  - aT  (transposed, for the score matmul):  fp8 e4m3, 4.2 MB/core.
    Scores only feed a softmax through tanh; fp8 here costs ~7e-3 rel
    err end-to-end (validated vs the fp32 reference, tolerance 2e-2).
    Batches 0-1 ship as singles (so scoring starts earlier), 2-7 as
    pairs (8 KB per partition row per DMA).
  - a_nat (natural, for the context matmul): bf16, 8.4 MB/core.  The
    last two batches ship as half-tiles so the final context matmuls
    can chase partial arrivals.
  Total 12.6 MB/core vs 16.8 MB for bf16-both.

DMA order = schedule (single Sync HWDGE stream; the DMA engines drain
descriptors in issue order): one small bf16 weight pack, then ALL aT,
then a_nat.  The scores+softmax pipeline completes while a_nat still
streams and the per-batch context matmuls chase the a_nat arrivals.

Score scatter (mm2) places each batch PAIR's e-rows at PSUM partition
bases 0/32/64/96 of ONE [98, Tx] tile via the matmul tile_position
column coordinate, so softmax needs only two full-width ACT passes
(tanh+exp over partitions [0:34] and [64:98]) and two 16-transpose
bursts — the first half's softmax weights are ready long before its
a_nat tiles arrive.  Relus: batches 0-3 slice 0 on ACT (free until the
first tanh), everything else on DVE tensor_scalar.

Context (per batch, a_nat-DMA-paced): ctx = sum_n p_n^T @ a_n as 4-way
column-tiled quads accumulating at PSUM partitions 0/32/64/96; the four
quarters leave PSUM as two 33-partition-wide DVE copies; host sums
quarters and divides by the denominator.

A small PE warm-up burst during the DMA lead-in starts the HAM clock
ramp early (the PE runs at half clock for the first ~10 us of work
regardless; the full-speed window is spent on the score pipeline).

Host-side preprocessing (transpose/cast/shard + final division) is numpy.
"""

import os
import sys

import numpy as np

for _p in ("/opt/trn_rl_repo", "/root/.axon_site/_ro/trn_rl_repo"):
    if os.path.isdir(_p) and _p not in sys.path:
        sys.path.insert(0, _p)

import ml_dtypes  # noqa: E402

import concourse.bacc as bacc  # noqa: E402
import concourse.bass as bass  # noqa: E402
import concourse.mybir as mybir  # noqa: E402
import concourse.tile as tile  # noqa: E402

BF16 = mybir.dt.bfloat16
F8 = mybir.dt.float8e4
F32 = mybir.dt.float32
NPBF16 = ml_dtypes.bfloat16
NPF8 = ml_dtypes.float8_e4m3
AF = mybir.ActivationFunctionType
ALU = mybir.AluOpType
PSUM = bass.MemorySpace.PSUM

NCORES = 8
B, TX, DA, DS, H = 64, 2048, 256, 256, 50
BPC = B // NCORES  # batches per core
NT = TX // 128  # 128-wide time chunks
NTS = TX // 512  # 512-wide time slices
KD = DA // 128  # contraction chunks over Da (and Ds)
NSING = 2  # leading at8 batches shipped as singles
NHALF = 2  # trailing a_nat batches shipped as half-tiles
PW = 34  # partition width of one softmax half (rows 0,1,32,33 live)

# Weight-pack column layout (single bf16 [128, 512] tensor, 1 KB rows).
_B_W1A = 0
_B_W1S = _B_W1A + KD * 64
_B_ST = _B_W1S + KD * H
_B_B1 = _B_ST + KD * BPC
_B_B2 = _B_B1 + 1
_B_W2 = _B_B2 + 1
_B_ID = _B_W2 + BPC * 2
WCOLS = 512


def jcol(bi):
    """Column of pT holding batch bi's weights within its softmax half."""
    return 32 * ((bi // 2) % 2) + bi % 2


def build_nc():
    """Build the (SPMD-identical) single-core Bass program."""
    nc = bacc.Bacc(
        "TRN2", target_bir_lowering=False, debug=False, num_devices=NCORES
    )

    NPAIR = (BPC - NSING) // 2
    at8a = nc.dram_tensor(
        "at8a", [NSING, 128, KD, TX], F8, kind="ExternalInput"
    )
    at8b = nc.dram_tensor(
        "at8b", [NPAIR, 128, 2, KD, TX], F8, kind="ExternalInput"
    )
    a_nat = nc.dram_tensor("a_nat", [BPC, 128, NT, DA], BF16, kind="ExternalInput")
    wpk = nc.dram_tensor("wpk", [128, WCOLS], BF16, kind="ExternalInput")
    ctx_o = nc.dram_tensor("ctx_o", [4, BPC, DA], F32, kind="ExternalOutput")
    den_o = nc.dram_tensor("den_o", [BPC, 1], F32, kind="ExternalOutput")

    with tile.TileContext(nc) as tc:
        with tc.tile_pool(name="const", bufs=1) as cpool, tc.tile_pool(
            name="at8s", bufs=NSING
        ) as atspool, tc.tile_pool(name="at8p", bufs=NPAIR) as atpool, tc.tile_pool(
            name="anat", bufs=BPC - NHALF
        ) as apool, tc.tile_pool(name="anath", bufs=2 * NHALF) as ahpool, tc.tile_pool(
            name="hsb", bufs=BPC
        ) as hsbp, tc.tile_pool(name="sb2", bufs=1) as sb2:
            ats_tiles = [
                atspool.tile([128, KD, TX], F8, name=f"ats{b}", tag="ats")
                for b in range(NSING)
            ]
            atp_tiles = [
                atpool.tile([128, 2, KD, TX], F8, name=f"atp{p}", tag="atp")
                for p in range(NPAIR)
            ]
            # a_nat: full tiles for b < BPC-NHALF, two half-tiles for the rest
            # (half h covers time chunks [8h, 8h+8)).
            a_tiles = {}
            for b in range(BPC - NHALF):
                a_tiles[b] = apool.tile(
                    [128, NT, DA], BF16, name=f"a_t{b}", tag="a_t"
                )
            for b in range(BPC - NHALF, BPC):
                a_tiles[b] = [
                    ahpool.tile([128, NT // 2, DA], BF16, name=f"a_t{b}h{h}",
                                tag="a_th")
                    for h in range(2)
                ]
            w = cpool.tile([128, WCOLS], BF16)
            nc.sync.dma_start(w[:], wpk[:])
            for b in range(NSING):
                nc.sync.dma_start(ats_tiles[b][:], at8a[b])
            for p in range(NPAIR):
                nc.sync.dma_start(atp_tiles[p][:], at8b[p])
            for b in range(BPC - NHALF):
                nc.sync.dma_start(a_tiles[b][:], a_nat[b])
            anat_h = a_nat.reshape([BPC, 128, 2, NT // 2, DA])
            for b in range(BPC - NHALF, BPC):
                for h in range(2):
                    nc.sync.dma_start(a_tiles[b][h][:], anat_h[b, :, h])

            def at_rhs(bi, k, ts):
                """Moving operand for mm1: one 512-wide time slice."""
                sl = slice(ts * 512, (ts + 1) * 512)
                if bi < NSING:
                    return ats_tiles[bi][:, k, sl]
                p, r = divmod(bi - NSING, 2)
                return atp_tiles[p][:, r, k, sl]

            def a_chunk(bi, n):
                """[128, DA] natural-layout chunk n of batch bi."""
                if bi < BPC - NHALF:
                    return a_tiles[bi][:, n, :]
                return a_tiles[bi][n // (NT // 2)][:, n % (NT // 2), :]

            sterm_sb = sb2.tile([128, BPC], F32)
            ctx_sb = sb2.tile([97, BPC, DA], F32)
            t_sb = sb2.tile([98, TX], F32, tag="tsb")
            p_sb = sb2.tile([98, TX], BF16, tag="psb")
            den_sb = sb2.tile([98, 1], F32, tag="den")
            warm_sb = sb2.tile([128, 512], BF16, tag="warm")
            nc.gpsimd.memset(warm_sb[:], 0.0)
            nc.gpsimd.memset(sterm_sb[:], 0.0)

            with tc.tile_pool(name="hps", bufs=2, space=PSUM) as hps, tc.tile_pool(
                name="eps", bufs=1, space=PSUM
            ) as eps, tc.tile_pool(name="p3", bufs=2, space=PSUM) as p3:
                # PE warm-up: short N=128 matmuls keep the PE continuously
                # busy through the DMA lead-in (weight pack + first aT tile)
                # so the HAM clock reaches full speed before the real score
                # matmuls start; otherwise the PE runs at half clock for the
                # first ~10 us of mm1.
                warm_ps = hps.tile([128, 512], F32, tag="hps", name="warm_ps")

                def emit_warm(n):
                    for _ in range(n):
                        nc.tensor.matmul(
                            warm_ps[0:64, 0:128],
                            warm_sb[:, 0:64],
                            warm_sb[:, 0:128],
                            start=True,
                            stop=True,
                            skip_group_check=True,
                        )

                emit_warm(14)

                # s-term, twice: partitions 0-49 (col group 0) and 64-113
                # (col group 64), so both relu halves get a bias.  Both
                # matmul groups land before either identity reads the tile
                # (a reader in between would WAR-stall the second group).
                sterm_ps = hps.tile([128, 512], F32, tag="hps", name="sterm_ps")
                for cg in (0, 64):
                    for k in range(KD):
                        nc.tensor.matmul(
                            sterm_ps[cg : cg + H, 0:BPC],
                            w[:, _B_W1S + k * H : _B_W1S + (k + 1) * H],
                            w[:, _B_ST + k * BPC : _B_ST + (k + 1) * BPC],
                            start=(k == 0),
                            stop=(k == KD - 1),
                            tile_position=(0, cg),
                            skip_group_check=True,
                        )
                for cg in (0, 64):
                    nc.scalar.activation(
                        sterm_sb[cg : cg + H, :],
                        sterm_ps[cg : cg + H, 0:BPC],
                        AF.Identity,
                        bias=w[cg : cg + H, _B_B1 : _B_B1 + 1],
                    )
                emit_warm(8)

                e_ps = eps.tile([98, TX], F32, tag="eps", name="e_ps")

                def emit_mm1(bi):
                    """Score matmuls + relus for one batch (column-tiled
                    512-wide pairs).  Returns the h tiles."""
                    tiles = []
                    for tp in range(NTS // 2):
                        h_ps = hps.tile([128, 512], F32, tag="hps")
                        for k in range(KD):
                            for half, cg in ((0, 0), (1, 64)):
                                ts = 2 * tp + half
                                nc.tensor.matmul(
                                    h_ps[cg : cg + 64, :],
                                    w[:, _B_W1A + k * 64 : _B_W1A + (k + 1) * 64],
                                    at_rhs(bi, k, ts),
                                    start=(k == 0),
                                    stop=(k == KD - 1),
                                    tile_position=(0, cg),
                                    skip_group_check=True,
                                )
                        h_sb = hsbp.tile([128, 512], BF16, tag="hsb")
                        if bi < 4 and tp == 0:
                            nc.scalar.activation(
                                h_sb[:],
                                h_ps[:],
                                AF.Relu,
                                bias=sterm_sb[:, bi : bi + 1],
                            )
                        else:
                            nc.vector.tensor_scalar(
                                h_sb[:],
                                h_ps[:],
                                sterm_sb[:, bi : bi + 1],
                                0.0,
                                ALU.add,
                                ALU.max,
                            )
                        tiles.append(h_sb)
                    return tiles

                def emit_mm2(bi, h_tiles):
                    """Scatter batch bi's scores into e_ps rows base+j via
                    the tile_position column coordinate."""
                    base = 32 * (bi // 2)
                    j = bi % 2
                    for tp in range(NTS // 2):
                        for half, cg in ((0, 0), (1, 64)):
                            ts = 2 * tp + half
                            nc.tensor.matmul(
                                e_ps[base : base + 2, ts * 512 : (ts + 1) * 512],
                                w[cg : cg + H, _B_W2 + bi * 2 : _B_W2 + (bi + 1) * 2],
                                h_tiles[tp][cg : cg + H, :],
                                start=(j == 0),
                                stop=(j == 1),
                                tile_position=(cg, base),
                                skip_group_check=True,
                            )

                def emit_softmax(hi):
                    """tanh -> exp(+den) over one 34-partition half on ACT."""
                    base = 64 * hi
                    sl = slice(base, base + PW)
                    nc.scalar.activation(
                        t_sb[sl, :],
                        e_ps[sl, :],
                        AF.Tanh,
                        bias=w[sl, _B_B2 : _B_B2 + 1],
                    )
                    nc.scalar.activation(
                        p_sb[sl, :],
                        t_sb[sl, :],
                        AF.Exp,
                        accum_out=den_sb[sl, :],
                    )
                    for g in (0, 1):
                        nc.gpsimd.dma_start(
                            den_o[4 * hi + 2 * g : 4 * hi + 2 * g + 2],
                            den_sb[base + 32 * g : base + 32 * g + 2, :],
                        )

                def emit_ptrans(hi):
                    """One 16-transpose burst: p rows [64hi : 64hi+34] ->
                    time-major pT, then a single DVE copy out."""
                    base = 64 * hi
                    pt_ps = p3.tile(
                        [128, NT * PW], BF16, tag="p3", name=f"pt_ps{hi}"
                    )
                    for n in range(NT):
                        nc.tensor.transpose(
                            pt_ps[:, n * PW : (n + 1) * PW],
                            p_sb[base : base + PW, n * 128 : (n + 1) * 128],
                            w[base : base + PW, _B_ID : _B_ID + PW],
                        )
                    pT_sb = sb2.tile([128, NT * PW], BF16, tag=f"pT{hi}")
                    nc.vector.tensor_copy(pT_sb[:], pt_ps[:])
                    return pT_sb

                def emit_ctx(bi, pT_sb):
                    jj = jcol(bi)
                    c_ps = p3.tile([128, DA], F32, tag="p3", name=f"c_ps{bi}")
                    for np_ in range(NT // 4):
                        for qi, cg in enumerate((0, 32, 64, 96)):
                            n = 4 * np_ + qi
                            nc.tensor.matmul(
                                c_ps[cg : cg + 1, :],
                                pT_sb[:, n * PW + jj : n * PW + jj + 1],
                                a_chunk(bi, n),
                                start=(np_ == 0),
                                stop=(np_ == NT // 4 - 1),
                                tile_position=(0, cg),
                                skip_group_check=True,
                            )
                    # Quarters sit at partitions 0/32/64/96; ship them as two
                    # 33-partition copies.  Later batches split DVE + ACT (ACT
                    # is reliably free once the second exp is done); the first
                    # three stay on DVE so a late exp can never stall them.
                    nc.vector.tensor_copy(ctx_sb[0:33, bi, :], c_ps[0:33, :])
                    if bi < 3:
                        nc.vector.tensor_copy(
                            ctx_sb[64:97, bi, :], c_ps[64:97, :]
                        )
                    else:
                        nc.scalar.copy(ctx_sb[64:97, bi, :], c_ps[64:97, :])

                def emit_out(lo):
                    engines = (nc.sync, nc.gpsimd, nc.scalar, nc.gpsimd)
                    for qi, cg in enumerate((0, 32, 64, 96)):
                        engines[qi].dma_start(
                            ctx_o[qi, lo : lo + 4, :],
                            ctx_sb[cg : cg + 1, lo : lo + 4, :],
                        )

                # ---- emission schedule ----
                # mm2 runs one batch behind mm1; each softmax half fires as
                # soon as its last mm2 is in; ctx(0-2) sit between the two
                # transpose bursts so they can chase the first a_nat arrivals.
                h_all = {}
                for bi in range(BPC):
                    h_all[bi] = emit_mm1(bi)
                    if bi > 0:
                        emit_mm2(bi - 1, h_all[bi - 1])
                    if bi == 4:
                        emit_softmax(0)
                emit_mm2(BPC - 1, h_all[BPC - 1])
                emit_softmax(1)
                pT_A = emit_ptrans(0)
                for bi in range(3):
                    emit_ctx(bi, pT_A)
                pT_B = emit_ptrans(1)
                emit_ctx(3, pT_A)
                emit_out(0)
                for bi in range(4, BPC):
                    emit_ctx(bi, pT_B)
                emit_out(4)

    nc.compile()
    return nc


def make_in_maps(a, s, W1, b1, W2, b2):
    a = np.asarray(a, np.float32)
    s = np.asarray(s, np.float32)
    W1 = np.asarray(W1, np.float32)
    b1 = np.asarray(b1, np.float32)
    W2 = np.asarray(W2, np.float32)
    b2 = np.asarray(b2, np.float32)

    NPAIR = (BPC - NSING) // 2
    a5 = a.reshape(NCORES, BPC, TX, DA)
    s3 = s.reshape(NCORES, BPC, DS)

    wpk_base = np.zeros((128, WCOLS), np.float32)
    w1a_full = np.zeros((128, KD, 64), np.float32)
    w1a_full[:, :, :H] = W1[:DA].reshape(KD, 128, H).transpose(1, 0, 2)
    wpk_base[:, _B_W1A : _B_W1A + KD * 64] = w1a_full.reshape(128, KD * 64)
    wpk_base[:, _B_W1S : _B_W1S + KD * H] = (
        W1[DA:].reshape(KD, 128, H).transpose(1, 0, 2).reshape(128, KD * H)
    )
    wpk_base[0:H, _B_B1] = b1
    wpk_base[64 : 64 + H, _B_B1] = b1
    wpk_base[:, _B_B2] = float(b2.reshape(-1)[0])
    oh = np.einsum(
        "h,bm->hbm", W2[:, 0], np.eye(2)[np.arange(BPC) % 2]
    ).reshape(H, BPC * 2)
    wpk_base[0:H, _B_W2 : _B_W2 + BPC * 2] = oh
    wpk_base[64 : 64 + H, _B_W2 : _B_W2 + BPC * 2] = oh
    wpk_base[0:PW, _B_ID : _B_ID + PW] = np.eye(PW)
    wpk_base[64 : 64 + PW, _B_ID : _B_ID + PW] = np.eye(PW)

    in_maps = []
    for i in range(NCORES):
        ai = a5[i]
        a_nat_h = np.ascontiguousarray(
            ai.reshape(BPC, NT, 128, DA).transpose(0, 2, 1, 3)
        ).astype(NPBF16)
        at_all = (
            ai.transpose(0, 2, 1).reshape(BPC, KD, 128, TX).transpose(0, 2, 1, 3)
        )
        at8a_h = np.ascontiguousarray(at_all[:NSING]).astype(NPF8)
        at8b_h = np.ascontiguousarray(
            at_all[NSING:].reshape(NPAIR, 2, 128, KD, TX).transpose(0, 2, 1, 3, 4)
        ).astype(NPF8)
        wpk_h = wpk_base.copy()
        wpk_h[:, _B_ST : _B_ST + KD * BPC] = (
            s3[i].T.reshape(KD, 128, BPC).transpose(1, 0, 2).reshape(128, KD * BPC)
        )
        in_maps.append(
            {
                "at8a": at8a_h,
                "at8b": at8b_h,
                "a_nat": a_nat_h,
                "wpk": wpk_h.astype(NPBF16),
            }
        )
    return in_maps


def assemble_output(results):
    outs = []
    for i in range(NCORES):
        ctx4 = results[i]["ctx_o"].astype(np.float64)
        ctx = ctx4.sum(axis=0)
        den = results[i]["den_o"].astype(np.float64)
        outs.append(ctx / den)
    return np.concatenate(outs, 0).reshape(B, 1, DA).astype(np.float32)


_NC_CACHE = None


def _get_nc():
    global _NC_CACHE
    if _NC_CACHE is None:
        _NC_CACHE = build_nc()
    return _NC_CACHE


def kernel(a, s, W1, b1, W2, b2, trace=False):
    from concourse.bass_utils import run_bass_kernel_spmd

    nc = _get_nc()
    in_maps = make_in_maps(a, s, W1, b1, W2, b2)
    res = run_bass_kernel_spmd(
        nc, in_maps, core_ids=list(range(NCORES)), trace=trace
    )
    out = assemble_output(res.results)
    if trace:
        kernel.last_exec_time_ns = res.exec_time_ns
        kernel.last_results = res
    return out


# revision 35
# speedup vs baseline: 1.1321x; 1.0712x over previous
"""Trainium2 Bass kernel for nn_AttentionLayer (Bahdanau-style attention scorer).

Math (per batch b):
    x   = concat([a, broadcast(s)], -1)            # [Tx, Da+Ds]
    h   = relu(x @ W1 + b1)                        # [Tx, H]
    e   = tanh(h @ W2 + b2)                        # [Tx, 1]
    al  = softmax(e, axis=Tx)
    ctx = al^T @ a                                 # [1, Da]

Since e = tanh(.) is in [-1, 1], softmax needs no max subtraction:
    al = exp(e) / sum(exp(e)) is numerically safe in fp32.

Sharding: data-parallel over B across 8 cores (8 batches each).

The kernel is HBM-bound, so `a` ships in mixed precision to cut bytes:
  - aT  (transposed, for the score matmul):  fp8 e4m3, 4.2 MB/core.
    Scores only feed a softmax through tanh; fp8 here costs ~7e-3 rel
    err end-to-end (validated vs the fp32 reference, tolerance 2e-2).
    Batches 0-1 ship as singles (so scoring starts earlier), 2-7 as
    pairs (8 KB per partition row per DMA).
  - a_nat (natural, for the context matmul): bf16, 8.4 MB/core.  The
    last two batches ship as half-tiles so the final context matmuls
    can chase partial arrivals.
  Total 12.6 MB/core vs 16.8 MB for bf16-both.

DMA order = schedule (single Sync HWDGE stream; the DMA engines drain
descriptors in issue order): one small bf16 weight pack, then ALL aT,
then a_nat.  The scores+softmax pipeline completes while a_nat still
streams and the per-batch context matmuls chase the a_nat arrivals.

Score scatter (mm2) places each batch PAIR's e-rows at PSUM partition
bases 0/32/64/96 of ONE [98, Tx] tile via the matmul tile_position
column coordinate, so softmax needs only two full-width ACT passes
(tanh+exp over partitions [0:34] and [64:98]) and two 16-transpose
bursts — the first half's softmax weights are ready long before its
a_nat tiles arrive.  Relus: batches 0-3 slice 0 on ACT (free until the
first tanh), everything else on DVE tensor_scalar.

Context (per batch, a_nat-DMA-paced): ctx = sum_n p_n^T @ a_n as 4-way
column-tiled quads accumulating at PSUM partitions 0/32/64/96; the four
quarters leave PSUM as two 33-partition-wide DVE copies; host sums
quarters and divides by the denominator.

A small PE warm-up burst during the DMA lead-in starts the HAM clock
ramp early (the PE runs at half clock for the first ~10 us of work
regardless; the full-speed window is spent on the score pipeline).

Host-side preprocessing (transpose/cast/shard + final division) is numpy.
"""

import os
import sys

import numpy as np

for _p in ("/opt/trn_rl_repo", "/root/.axon_site/_ro/trn_rl_repo"):
    if os.path.isdir(_p) and _p not in sys.path:
        sys.path.insert(0, _p)

import ml_dtypes  # noqa: E402

import concourse.bacc as bacc  # noqa: E402
import concourse.bass as bass  # noqa: E402
import concourse.mybir as mybir  # noqa: E402
import concourse.tile as tile  # noqa: E402

BF16 = mybir.dt.bfloat16
F8 = mybir.dt.float8e4
F32 = mybir.dt.float32
NPBF16 = ml_dtypes.bfloat16
NPF8 = ml_dtypes.float8_e4m3
AF = mybir.ActivationFunctionType
ALU = mybir.AluOpType
PSUM = bass.MemorySpace.PSUM

NCORES = 8
B, TX, DA, DS, H = 64, 2048, 256, 256, 50
BPC = B // NCORES  # batches per core
NT = TX // 128  # 128-wide time chunks
NTS = TX // 512  # 512-wide time slices
KD = DA // 128  # contraction chunks over Da (and Ds)
NSING = 2  # leading at8 batches shipped as singles
NHALF = 2  # trailing a_nat batches shipped as half-tiles
PW = 34  # partition width of one softmax half (rows 0,1,32,33 live)

# Weight-pack column layout (single bf16 [128, 512] tensor, 1 KB rows).
_B_W1A = 0
_B_W1S = _B_W1A + KD * 64
_B_ST = _B_W1S + KD * H
_B_B1 = _B_ST + KD * BPC
_B_B2 = _B_B1 + 1
_B_W2 = _B_B2 + 1
_B_ID = _B_W2 + BPC * 2
WCOLS = 512


def jcol(bi):
    """Column of pT holding batch bi's weights within its softmax half."""
    return 32 * ((bi // 2) % 2) + bi % 2


def build_nc():
    """Build the (SPMD-identical) single-core Bass program."""
    nc = bacc.Bacc(
        "TRN2", target_bir_lowering=False, debug=False, num_devices=NCORES
    )

    NPAIR = (BPC - NSING) // 2
    at8a = nc.dram_tensor(
        "at8a", [NSING, 128, KD, TX], F8, kind="ExternalInput"
    )
    at8b = nc.dram_tensor(
        "at8b", [NPAIR, 128, 2, KD, TX], F8, kind="ExternalInput"
    )
    a_nat = nc.dram_tensor("a_nat", [BPC, 128, NT, DA], BF16, kind="ExternalInput")
    wpk = nc.dram_tensor("wpk", [128, WCOLS], BF16, kind="ExternalInput")
    ctx_o = nc.dram_tensor("ctx_o", [4, BPC, DA], F32, kind="ExternalOutput")
    den_o = nc.dram_tensor("den_o", [BPC, 1], F32, kind="ExternalOutput")

    with tile.TileContext(nc) as tc:
        with tc.tile_pool(name="const", bufs=1) as cpool, tc.tile_pool(
            name="at8s", bufs=NSING
        ) as atspool, tc.tile_pool(name="at8p", bufs=NPAIR) as atpool, tc.tile_pool(
            name="anat", bufs=BPC - NHALF
        ) as apool, tc.tile_pool(name="anath", bufs=6) as ahpool, tc.tile_pool(
            name="hsb", bufs=BPC
        ) as hsbp, tc.tile_pool(name="sb2", bufs=1) as sb2:
            ats_tiles = [
                atspool.tile([128, KD, TX], F8, name=f"ats{b}", tag="ats")
                for b in range(NSING)
            ]
            atp_tiles = [
                atpool.tile([128, 2, KD, TX], F8, name=f"atp{p}", tag="atp")
                for p in range(NPAIR)
            ]
            # a_nat: full tiles early; batch 6 in halves and batch 7 in
            # quarters so the tail context matmuls chase partial arrivals
            # (quarter q covers exactly quad-group np_ = q).
            nparts = {BPC - 2: 2, BPC - 1: 4}
            a_tiles = {}
            for b in range(BPC - NHALF):
                a_tiles[b] = apool.tile(
                    [128, NT, DA], BF16, name=f"a_t{b}", tag="a_t"
                )
            for b in range(BPC - NHALF, BPC):
                np_b = nparts[b]
                a_tiles[b] = [
                    ahpool.tile([128, NT // np_b, DA], BF16,
                                name=f"a_t{b}h{h}", tag="a_th")
                    for h in range(np_b)
                ]
            w = cpool.tile([128, WCOLS], BF16)
            nc.sync.dma_start(w[:], wpk[:])
            for b in range(NSING):
                nc.sync.dma_start(ats_tiles[b][:], at8a[b])
            for p in range(NPAIR):
                nc.sync.dma_start(atp_tiles[p][:], at8b[p])
            for b in range(BPC - NHALF):
                nc.sync.dma_start(a_tiles[b][:], a_nat[b])
            for b in range(BPC - NHALF, BPC):
                np_b = nparts[b]
                anat_v = a_nat.reshape([BPC, 128, np_b, NT // np_b, DA])
                for h in range(np_b):
                    nc.sync.dma_start(a_tiles[b][h][:], anat_v[b, :, h])

            def at_rhs(bi, k, ts):
                """Moving operand for mm1: one 512-wide time slice."""
                sl = slice(ts * 512, (ts + 1) * 512)
                if bi < NSING:
                    return ats_tiles[bi][:, k, sl]
                p, r = divmod(bi - NSING, 2)
                return atp_tiles[p][:, r, k, sl]

            def a_chunk(bi, n):
                """[128, DA] natural-layout chunk n of batch bi."""
                if bi < BPC - NHALF:
                    return a_tiles[bi][:, n, :]
                per = NT // nparts[bi]
                return a_tiles[bi][n // per][:, n % per, :]

            sterm_sb = sb2.tile([128, BPC], F32)
            ctx_sb = sb2.tile([97, BPC, DA], F32)
            t_sb = sb2.tile([98, TX], F32, tag="tsb")
            p_sb = sb2.tile([98, TX], BF16, tag="psb")
            den_sb = sb2.tile([98, 1], F32, tag="den")
            warm_sb = sb2.tile([128, 512], BF16, tag="warm")
            nc.gpsimd.memset(warm_sb[:], 0.0)
            nc.gpsimd.memset(sterm_sb[:], 0.0)

            with tc.tile_pool(name="hps", bufs=2, space=PSUM) as hps, tc.tile_pool(
                name="eps", bufs=1, space=PSUM
            ) as eps, tc.tile_pool(name="p3", bufs=2, space=PSUM) as p3:
                # PE warm-up: short N=128 matmuls keep the PE continuously
                # busy through the DMA lead-in (weight pack + first aT tile)
                # so the HAM clock reaches full speed before the real score
                # matmuls start; otherwise the PE runs at half clock for the
                # first ~10 us of mm1.
                warm_ps = hps.tile([128, 512], F32, tag="hps", name="warm_ps")

                def emit_warm(n):
                    for _ in range(n):
                        nc.tensor.matmul(
                            warm_ps[0:64, 0:128],
                            warm_sb[:, 0:64],
                            warm_sb[:, 0:128],
                            start=True,
                            stop=True,
                            skip_group_check=True,
                        )

                emit_warm(14)

                # s-term, twice: partitions 0-49 (col group 0) and 64-113
                # (col group 64), so both relu halves get a bias.  Both
                # matmul groups land before either identity reads the tile
                # (a reader in between would WAR-stall the second group).
                sterm_ps = hps.tile([128, 512], F32, tag="hps", name="sterm_ps")
                for cg in (0, 64):
                    for k in range(KD):
                        nc.tensor.matmul(
                            sterm_ps[cg : cg + H, 0:BPC],
                            w[:, _B_W1S + k * H : _B_W1S + (k + 1) * H],
                            w[:, _B_ST + k * BPC : _B_ST + (k + 1) * BPC],
                            start=(k == 0),
                            stop=(k == KD - 1),
                            tile_position=(0, cg),
                            skip_group_check=True,
                        )
                for cg in (0, 64):
                    nc.scalar.activation(
                        sterm_sb[cg : cg + H, :],
                        sterm_ps[cg : cg + H, 0:BPC],
                        AF.Identity,
                        bias=w[cg : cg + H, _B_B1 : _B_B1 + 1],
                    )
                emit_warm(8)

                e_ps = eps.tile([98, TX], F32, tag="eps", name="e_ps")

                def emit_mm1(bi):
                    """Score matmuls + relus for one batch (column-tiled
                    512-wide pairs).  Returns the h tiles."""
                    tiles = []
                    for tp in range(NTS // 2):
                        h_ps = hps.tile([128, 512], F32, tag="hps")
                        for k in range(KD):
                            for half, cg in ((0, 0), (1, 64)):
                                ts = 2 * tp + half
                                nc.tensor.matmul(
                                    h_ps[cg : cg + 64, :],
                                    w[:, _B_W1A + k * 64 : _B_W1A + (k + 1) * 64],
                                    at_rhs(bi, k, ts),
                                    start=(k == 0),
                                    stop=(k == KD - 1),
                                    tile_position=(0, cg),
                                    skip_group_check=True,
                                )
                        h_sb = hsbp.tile([128, 512], BF16, tag="hsb")
                        if bi < 4 and tp == 0:
                            nc.scalar.activation(
                                h_sb[:],
                                h_ps[:],
                                AF.Relu,
                                bias=sterm_sb[:, bi : bi + 1],
                            )
                        else:
                            nc.vector.tensor_scalar(
                                h_sb[:],
                                h_ps[:],
                                sterm_sb[:, bi : bi + 1],
                                0.0,
                                ALU.add,
                                ALU.max,
                            )
                        tiles.append(h_sb)
                    return tiles

                def emit_mm2(bi, h_tiles):
                    """Scatter batch bi's scores into e_ps rows base+j via
                    the tile_position column coordinate."""
                    base = 32 * (bi // 2)
                    j = bi % 2
                    for tp in range(NTS // 2):
                        for half, cg in ((0, 0), (1, 64)):
                            ts = 2 * tp + half
                            nc.tensor.matmul(
                                e_ps[base : base + 2, ts * 512 : (ts + 1) * 512],
                                w[cg : cg + H, _B_W2 + bi * 2 : _B_W2 + (bi + 1) * 2],
                                h_tiles[tp][cg : cg + H, :],
                                start=(j == 0),
                                stop=(j == 1),
                                tile_position=(cg, base),
                                skip_group_check=True,
                            )

                def emit_softmax(hi):
                    """tanh -> exp(+den) over one 34-partition half on ACT."""
                    base = 64 * hi
                    sl = slice(base, base + PW)
                    nc.scalar.activation(
                        t_sb[sl, :],
                        e_ps[sl, :],
                        AF.Tanh,
                        bias=w[sl, _B_B2 : _B_B2 + 1],
                    )
                    nc.scalar.activation(
                        p_sb[sl, :],
                        t_sb[sl, :],
                        AF.Exp,
                        accum_out=den_sb[sl, :],
                    )
                    for g in (0, 1):
                        nc.gpsimd.dma_start(
                            den_o[4 * hi + 2 * g : 4 * hi + 2 * g + 2],
                            den_sb[base + 32 * g : base + 32 * g + 2, :],
                        )

                def emit_ptrans(hi):
                    """One 16-transpose burst: p rows [64hi : 64hi+34] ->
                    time-major pT, then a single DVE copy out."""
                    base = 64 * hi
                    pt_ps = p3.tile(
                        [128, NT * PW], BF16, tag="p3", name=f"pt_ps{hi}"
                    )
                    for n in range(NT):
                        nc.tensor.transpose(
                            pt_ps[:, n * PW : (n + 1) * PW],
                            p_sb[base : base + PW, n * 128 : (n + 1) * 128],
                            w[base : base + PW, _B_ID : _B_ID + PW],
                        )
                    pT_sb = sb2.tile([128, NT * PW], BF16, tag=f"pT{hi}")
                    nc.vector.tensor_copy(pT_sb[:], pt_ps[:])
                    return pT_sb

                def emit_ctx(bi, pT_sb):
                    jj = jcol(bi)
                    c_ps = p3.tile([128, DA], F32, tag="p3", name=f"c_ps{bi}")
                    for np_ in range(NT // 4):
                        for qi, cg in enumerate((0, 32, 64, 96)):
                            n = 4 * np_ + qi
                            nc.tensor.matmul(
                                c_ps[cg : cg + 1, :],
                                pT_sb[:, n * PW + jj : n * PW + jj + 1],
                                a_chunk(bi, n),
                                start=(np_ == 0),
                                stop=(np_ == NT // 4 - 1),
                                tile_position=(0, cg),
                                skip_group_check=True,
                            )
                    # Quarters sit at partitions 0/32/64/96; ship them as two
                    # 33-partition copies.  Later batches split DVE + ACT (ACT
                    # is reliably free once the second exp is done); the first
                    # three stay on DVE so a late exp can never stall them.
                    nc.vector.tensor_copy(ctx_sb[0:33, bi, :], c_ps[0:33, :])
                    if bi < 3:
                        nc.vector.tensor_copy(
                            ctx_sb[64:97, bi, :], c_ps[64:97, :]
                        )
                    else:
                        nc.scalar.copy(ctx_sb[64:97, bi, :], c_ps[64:97, :])

                def emit_out(lo, n):
                    engines = (nc.sync, nc.gpsimd, nc.scalar, nc.gpsimd)
                    if lo + n == BPC:  # tail: keep off the slow SWDGE path
                        engines = (nc.sync, nc.scalar, nc.sync, nc.scalar)
                    for qi, cg in enumerate((0, 32, 64, 96)):
                        engines[qi].dma_start(
                            ctx_o[qi, lo : lo + n, :],
                            ctx_sb[cg : cg + 1, lo : lo + n, :],
                        )

                # ---- emission schedule ----
                # mm2 runs one batch behind mm1; each softmax half fires as
                # soon as its last mm2 is in; ctx(0-2) sit between the two
                # transpose bursts so they can chase the first a_nat arrivals.
                h_all = {}
                for bi in range(BPC):
                    h_all[bi] = emit_mm1(bi)
                    if bi > 0:
                        emit_mm2(bi - 1, h_all[bi - 1])
                    if bi == 4:
                        emit_softmax(0)
                emit_mm2(BPC - 1, h_all[BPC - 1])
                emit_softmax(1)
                pT_A = emit_ptrans(0)
                for bi in range(3):
                    emit_ctx(bi, pT_A)
                pT_B = emit_ptrans(1)
                emit_ctx(3, pT_A)
                emit_out(0, 4)
                emit_ctx(4, pT_B)
                emit_ctx(5, pT_B)
                emit_out(4, 2)
                emit_ctx(6, pT_B)
                emit_ctx(7, pT_B)
                emit_out(6, 2)

    nc.compile()
    return nc


def make_in_maps(a, s, W1, b1, W2, b2):
    a = np.asarray(a, np.float32)
    s = np.asarray(s, np.float32)
    W1 = np.asarray(W1, np.float32)
    b1 = np.asarray(b1, np.float32)
    W2 = np.asarray(W2, np.float32)
    b2 = np.asarray(b2, np.float32)

    NPAIR = (BPC - NSING) // 2
    a5 = a.reshape(NCORES, BPC, TX, DA)
    s3 = s.reshape(NCORES, BPC, DS)

    wpk_base = np.zeros((128, WCOLS), np.float32)
    w1a_full = np.zeros((128, KD, 64), np.float32)
    w1a_full[:, :, :H] = W1[:DA].reshape(KD, 128, H).transpose(1, 0, 2)
    wpk_base[:, _B_W1A : _B_W1A + KD * 64] = w1a_full.reshape(128, KD * 64)
    wpk_base[:, _B_W1S : _B_W1S + KD * H] = (
        W1[DA:].reshape(KD, 128, H).transpose(1, 0, 2).reshape(128, KD * H)
    )
    wpk_base[0:H, _B_B1] = b1
    wpk_base[64 : 64 + H, _B_B1] = b1
    wpk_base[:, _B_B2] = float(b2.reshape(-1)[0])
    oh = np.einsum(
        "h,bm->hbm", W2[:, 0], np.eye(2)[np.arange(BPC) % 2]
    ).reshape(H, BPC * 2)
    wpk_base[0:H, _B_W2 : _B_W2 + BPC * 2] = oh
    wpk_base[64 : 64 + H, _B_W2 : _B_W2 + BPC * 2] = oh
    wpk_base[0:PW, _B_ID : _B_ID + PW] = np.eye(PW)
    wpk_base[64 : 64 + PW, _B_ID : _B_ID + PW] = np.eye(PW)

    in_maps = []
    for i in range(NCORES):
        ai = a5[i]
        a_nat_h = np.ascontiguousarray(
            ai.reshape(BPC, NT, 128, DA).transpose(0, 2, 1, 3)
        ).astype(NPBF16)
        at_all = (
            ai.transpose(0, 2, 1).reshape(BPC, KD, 128, TX).transpose(0, 2, 1, 3)
        )
        at8a_h = np.ascontiguousarray(at_all[:NSING]).astype(NPF8)
        at8b_h = np.ascontiguousarray(
            at_all[NSING:].reshape(NPAIR, 2, 128, KD, TX).transpose(0, 2, 1, 3, 4)
        ).astype(NPF8)
        wpk_h = wpk_base.copy()
        wpk_h[:, _B_ST : _B_ST + KD * BPC] = (
            s3[i].T.reshape(KD, 128, BPC).transpose(1, 0, 2).reshape(128, KD * BPC)
        )
        in_maps.append(
            {
                "at8a": at8a_h,
                "at8b": at8b_h,
                "a_nat": a_nat_h,
                "wpk": wpk_h.astype(NPBF16),
            }
        )
    return in_maps


def assemble_output(results):
    outs = []
    for i in range(NCORES):
        ctx4 = results[i]["ctx_o"].astype(np.float64)
        ctx = ctx4.sum(axis=0)
        den = results[i]["den_o"].astype(np.float64)
        outs.append(ctx / den)
    return np.concatenate(outs, 0).reshape(B, 1, DA).astype(np.float32)


_NC_CACHE = None


def _get_nc():
    global _NC_CACHE
    if _NC_CACHE is None:
        _NC_CACHE = build_nc()
    return _NC_CACHE


def kernel(a, s, W1, b1, W2, b2, trace=False):
    from concourse.bass_utils import run_bass_kernel_spmd

    nc = _get_nc()
    in_maps = make_in_maps(a, s, W1, b1, W2, b2)
    res = run_bass_kernel_spmd(
        nc, in_maps, core_ids=list(range(NCORES)), trace=trace
    )
    out = assemble_output(res.results)
    if trace:
        kernel.last_exec_time_ns = res.exec_time_ns
        kernel.last_results = res
    return out
